# revision 1
# baseline (speedup 1.0000x reference)
"""Compositional attention Trainium2 Bass kernel (V2: bf16 matmul path).

Sharding: 8 cores = 2 batches x 4 search-pairs.
Core c handles batch b=c//4 and searches (2*(c%4), 2*(c%4)+1); each core
produces a partial output for its 128 columns of the S*D=512 concat dim
(host sums 4 partials per batch).

V2 notes:
  - All hot matmuls run in bf16 (fp32 matmuls are split into 2 passes by
    the compiler = half throughput) with 1024-wide moving operands.
  - Softmax denominators come from a DVE pairwise add tree over the
    exp'd score tiles + one ones-matmul partition reduce (the V1
    per-key-tile ones-matmuls were ~25% of PE time).
  - Normalization is folded into the R-softmax/combine epilogue (exact).
"""

import sys

for _p in ("/opt/trn_rl_repo",):
    if _p not in sys.path:
        sys.path.insert(0, _p)

from contextlib import ExitStack

import ml_dtypes
import numpy as np

import concourse.bass as bass
import concourse.tile as tile
from concourse import bacc
from concourse import mybir
from concourse.bass import ts
from concourse.bass_utils import run_bass_kernel_spmd
from concourse.masks import make_identity

B, N, DIM, S, R, D = 2, 2048, 1024, 8, 2, 64
NCORES = 8
SPC = 2          # searches per core
SD = SPC * D     # 128 (per-core slice of S*D)
RD = R * D       # 128
P = 128
IBL = 512        # i-block (query block)
NIB = N // IBL   # 4
EBL = 512        # epilogue fp32 matmul free dim
NEB = N // EBL   # 4
KC = DIM // P    # 8
NJT = N // P     # 16 key tiles
F32 = mybir.dt.float32
BF16 = mybir.dt.bfloat16
SCALE = float(D) ** -0.5
AF = mybir.ActivationFunctionType
ALU = mybir.AluOpType


def _emit(ctx: ExitStack, tc: tile.TileContext, io):
    nc = tc.nc
    xT, wq, wk, wr, wv, wrk, wout, outp = io

    singles = ctx.enter_context(tc.tile_pool(name="singles", bufs=1))
    ident = singles.tile([P, P], BF16)
    make_identity(nc, ident)
    ones_b = singles.tile([P, 1], BF16)
    nc.vector.memset(ones_b, 1.0)
    ones_f = singles.tile([P, 1], F32)
    nc.vector.memset(ones_f, 1.0)

    wq_sb = singles.tile([P, KC, SD], BF16)
    wk_sb = singles.tile([P, KC, SD], BF16)
    wr_sb = singles.tile([P, KC, SD], BF16)
    wv_sb = singles.tile([P, KC, RD], BF16)
    for dst, src in ((wq_sb, wq), (wk_sb, wk), (wr_sb, wr), (wv_sb, wv)):
        nc.sync.dma_start(out=dst, in_=src.rearrange("(kc p) m -> p kc m", p=P))
    wrk_sb = singles.tile([D, D], F32)
    nc.sync.dma_start(out=wrk_sb, in_=wrk)
    wout_sb = singles.tile([P, DIM], BF16)
    nc.sync.dma_start(out=wout_sb, in_=wout)

    acts = ctx.enter_context(tc.tile_pool(name="acts", bufs=1))
    qT = acts.tile([P, N], BF16)
    kT = acts.tile([P, N], BF16)
    rqT = acts.tile([P, N], F32)
    vT = acts.tile([P, N], BF16)
    vnat = acts.tile([P, NJT, RD], BF16)   # [key-part, key-tile, r*d]
    ret0 = acts.tile([P, N], F32)          # search0 retrievedT (unnormalized)
    ret1 = acts.tile([P, N], F32)          # search1
    rq_lo = acts.tile([64, N], F32)        # search1 rq realigned to parts 0:64
    comp = acts.tile([P, N], BF16)          # composed output, stacked searches
    red0 = acts.tile([P, N], BF16)         # per-key-tile exp partial sums
    red1 = acts.tile([P, N], BF16)

    # ---------------- projections ----------------
    with tc.tile_pool(name="xpool", bufs=1) as xpool, \
         tc.tile_pool(name="ppsum", bufs=3, space="PSUM") as ppsum, \
         tc.tile_pool(name="tpsum", bufs=2, space="PSUM") as tpsum:
        xs = xpool.tile([P, KC, N], BF16)
        nc.sync.dma_start(out=xs, in_=xT.rearrange("(kc p) n -> p kc n", p=P))
        for wsb, dest in ((wq_sb, qT), (wk_sb, kT), (wr_sb, rqT), (wv_sb, vT)):
            pss = [ppsum.tile([P, IBL], F32, tag="pj", name=f"pj{ib}")
                   for ib in range(NIB)]
            for k in range(KC):
                for ib in range(NIB):
                    nc.tensor.matmul(
                        pss[ib],
                        lhsT=wsb[:, k, :],
                        rhs=xs[:, k, ts(ib, IBL)],
                        start=(k == 0),
                        stop=(k == KC - 1),
                    )
            for ib in range(NIB):
                nc.vector.tensor_copy(out=dest[:, ts(ib, IBL)], in_=pss[ib])
        # v to natural [keys, r*d] layout via PE transpose (bf16, single pass)
        for jt in range(NJT):
            tp = tpsum.tile([P, P], BF16, tag="tp")
            nc.tensor.transpose(tp, vT[:, ts(jt, P)], ident)
            nc.vector.tensor_copy(out=vnat[:, jt, :], in_=tp)
        nc.gpsimd.dma_start(out=rq_lo, in_=rqT[64:128, :])

    # DRAM bounce buffers for per-query scalars ([1,N] <-> [128,N/128] dances)
    dramp = ctx.enter_context(tc.tile_pool(name="dramp", bufs=1, space="DRAM"))
    diff_dr = [dramp.tile([N], F32, tag=f"diff{si}", name=f"diff{si}")
               for si in range(SPC)]
    sums_dr = [dramp.tile([N], F32, tag=f"sums{si}", name=f"sums{si}")
               for si in range(SPC)]
    ra0_dr = [dramp.tile([N], F32, tag=f"ra0{si}", name=f"ra0d{si}")
              for si in range(SPC)]
    ra1_dr = [dramp.tile([N], F32, tag=f"ra1{si}", name=f"ra1d{si}")
              for si in range(SPC)]

    rets = (ret0, ret1)
    reds = (red0, red1)

    # ---------------- attention ----------------
    with tc.tile_pool(name="expp", bufs=2) as expp, \
         tc.tile_pool(name="trp0", bufs=2) as trp0, \
         tc.tile_pool(name="trp1", bufs=2) as trp1, \
         tc.tile_pool(name="trp2", bufs=2) as trp2, \
         tc.tile_pool(name="scp", bufs=2, space="PSUM") as scp, \
         tc.tile_pool(name="mps", bufs=2, space="PSUM") as mps:
        for ib in range(NIB):
            ets = [expp.tile([P, NJT, IBL], BF16, tag="exp", name=f"exp{si}")
                   for si in range(SPC)]
            rt = [mps.tile([P, IBL], F32, tag="mm", name=f"rt{si}")
                  for si in range(SPC)]
            # scores -> exp -> retrieval, interleaved per key-tile pair so the
            # PE always has independent matmuls in flight (HAM stays warm)
            for jg in range(NJT // 2):
                for si in range(SPC):
                    lo = 64 * si
                    sp = scp.tile([P, 2, IBL], F32, tag="sc", name=f"sc{si}")
                    for h in range(2):
                        jt = 2 * jg + h
                        nc.tensor.matmul(
                            sp[:, h, :],
                            lhsT=kT[lo:lo + 64, ts(jt, P)],
                            rhs=qT[lo:lo + 64, ts(ib, IBL)],
                            start=True, stop=True,
                        )
                    nc.scalar.activation(
                        out=ets[si][:, ts(jg, 2), :], in_=sp,
                        func=AF.Exp, scale=SCALE,
                    )
                    for h in range(2):
                        jt = 2 * jg + h
                        nc.tensor.matmul(
                            rt[si], lhsT=vnat[:, jt, :], rhs=ets[si][:, jt, :],
                            start=(jt == 0), stop=(jt == NJT - 1),
                            skip_group_check=True,
                        )
            for si in range(SPC):
                nc.vector.tensor_copy(out=rets[si][:, ts(ib, IBL)], in_=rt[si])
            # wide add tree over key tiles (split DVE / GpSimd)
            for si in range(SPC):
                g1 = trp0.tile([P, NJT // 2, IBL], BF16, tag="g1",
                               name=f"g1_{si}")
                nc.vector.tensor_tensor(g1, ets[si][:, 0:8, :],
                                        ets[si][:, 8:16, :], ALU.add)
                g2 = trp1.tile([P, NJT // 4, IBL], BF16, tag="g2",
                               name=f"g2_{si}")
                nc.gpsimd.tensor_tensor(g2, g1[:, 0:4, :], g1[:, 4:8, :],
                                        ALU.add)
                g3 = trp2.tile([P, NJT // 8, IBL], BF16, tag="g3",
                               name=f"g3_{si}")
                nc.vector.tensor_tensor(g3, g2[:, 0:2, :], g2[:, 2:4, :],
                                        ALU.add)
                nc.gpsimd.tensor_tensor(reds[si][:, ts(ib, IBL)],
                                        g3[:, 0, :], g3[:, 1, :], ALU.add)

    # ---------------- epilogue ----------------
    eps = ctx.enter_context(tc.tile_pool(name="eps", bufs=4, space="PSUM"))
    etmp = ctx.enter_context(tc.tile_pool(name="etmp", bufs=2))
    btmp = ctx.enter_context(tc.tile_pool(name="btmp", bufs=1))
    for si in range(SPC):
        retX = rets[si]
        # partition-reduce of per-key-tile sums -> softmax denominators
        for ib in range(NIB):
            psm = eps.tile([P, IBL], F32, tag="ep", name="psm")
            nc.tensor.matmul(psm[:1, :], lhsT=ones_b,
                             rhs=reds[si][:, ts(ib, IBL)], start=True,
                             stop=True)
            smst = etmp.tile([1, IBL], F32, tag="smst")
            nc.vector.tensor_copy(out=smst, in_=psm[:1, :])
            nc.sync.dma_start(out=sums_dr[si][None, ts(ib, IBL)],
                              in_=smst[0:1, :])

        r1lo = btmp.tile([64, N], F32, tag="r1lo")
        nc.gpsimd.dma_start(out=r1lo, in_=retX[64:128, :])
        rqs = rqT[0:64, :] if si == 0 else rq_lo

        dprod = btmp.tile([64, N], F32, tag="dprod")
        for ib in range(NEB):
            pk0 = eps.tile([P, IBL], F32, tag="ep", name="pk0")
            pk1 = eps.tile([P, IBL], F32, tag="ep", name="pk1")
            nc.tensor.matmul(pk0[:64, :EBL], lhsT=wrk_sb,
                             rhs=retX[0:64, ts(ib, EBL)], start=True, stop=True)
            nc.tensor.matmul(pk1[:64, :EBL], lhsT=wrk_sb,
                             rhs=r1lo[:, ts(ib, EBL)], start=True, stop=True)
            rk0s = etmp.tile([64, EBL], F32, tag="rk0")
            nc.vector.tensor_copy(out=rk0s, in_=pk0[:64, :EBL])
            dsub = etmp.tile([64, EBL], F32, tag="dsub")
            nc.vector.tensor_tensor(dsub, rk0s, pk1[:64, :EBL], ALU.subtract)
            nc.vector.tensor_tensor(dprod[:, ts(ib, EBL)],
                                    rqs[:, ts(ib, EBL)], dsub, ALU.mult)
        for ib in range(NEB):
            pd = eps.tile([P, IBL], F32, tag="ep", name="pd")
            nc.tensor.matmul(pd[:1, :EBL], lhsT=ones_f[0:64, :],
                             rhs=dprod[:, ts(ib, EBL)], start=True, stop=True)
            pdst = etmp.tile([1, EBL], F32, tag="pdst")
            nc.vector.tensor_copy(out=pdst, in_=pd[:1, :EBL])
            nc.sync.dma_start(out=diff_dr[si][None, ts(ib, EBL)],
                              in_=pdst[0:1, :])

        d128 = etmp.tile([P, N // P], F32, tag="d128")
        s128 = etmp.tile([P, N // P], F32, tag="s128")
        nc.gpsimd.dma_start(out=d128,
                            in_=diff_dr[si].rearrange("(p f) -> p f", p=P))
        nc.gpsimd.dma_start(out=s128,
                            in_=sums_dr[si].rearrange("(p f) -> p f", p=P))
        inv = etmp.tile([P, N // P], F32, tag="inv")
        nc.vector.reciprocal(inv, s128)
        t16 = etmp.tile([P, N // P], F32, tag="t16")
        nc.vector.tensor_tensor(t16, d128, inv, ALU.mult)
        ra0 = etmp.tile([P, N // P], F32, tag="ra0")
        nc.scalar.activation(out=ra0, in_=t16, func=AF.Sigmoid, scale=SCALE)
        ra0s = etmp.tile([P, N // P], F32, tag="ra0s")
        nc.vector.tensor_tensor(ra0s, ra0, inv, ALU.mult)
        ra1s = etmp.tile([P, N // P], F32, tag="ra1s")
        nc.vector.tensor_tensor(ra1s, inv, ra0s, ALU.subtract)
        nc.gpsimd.dma_start(out=ra0_dr[si].rearrange("(p f) -> p f", p=P),
                            in_=ra0s)
        nc.gpsimd.dma_start(out=ra1_dr[si].rearrange("(p f) -> p f", p=P),
                            in_=ra1s)

        bra0 = btmp.tile([64, N], F32, tag="bra0")
        bra1 = btmp.tile([64, N], F32, tag="bra1")
        nc.gpsimd.dma_start(out=bra0,
                            in_=ra0_dr[si][None, :].to_broadcast([64, N]))
        nc.gpsimd.dma_start(out=bra1,
                            in_=ra1_dr[si][None, :].to_broadcast([64, N]))
        t1 = btmp.tile([64, N], F32, tag="t1")
        t2 = btmp.tile([64, N], F32, tag="t2")
        nc.vector.tensor_tensor(t1, bra0, retX[0:64, :], ALU.mult)
        nc.vector.tensor_tensor(t2, bra1, r1lo, ALU.mult)
        if si == 0:
            nc.vector.tensor_tensor(comp[0:64, :], t1, t2, ALU.add)
        else:
            cs1 = btmp.tile([64, N], F32, tag="cs1")
            nc.vector.tensor_tensor(cs1, t1, t2, ALU.add)
            nc.gpsimd.dma_start(out=comp[64:128, :], in_=cs1)

    # ---------------- output projection ----------------
    for nch in range(N // P):
        for h in range(DIM // EBL):
            pw = eps.tile([P, IBL], F32, tag="ep", name="pw")
            nc.tensor.matmul(pw[:, :EBL], lhsT=comp[:, ts(nch, P)],
                             rhs=wout_sb[:, ts(h, EBL)], start=True, stop=True)
            owst = etmp.tile([P, EBL], F32, tag="owst")
            nc.any.tensor_copy(out=owst, in_=pw[:, :EBL])
            nc.sync.dma_start(out=outp[ts(nch, P), ts(h, EBL)], in_=owst)


def build_nc():
    nc = bacc.Bacc()
    xT = nc.declare_dram_parameter("xT", [DIM, N], BF16, isOutput=False)
    wq = nc.declare_dram_parameter("wq", [DIM, SD], BF16, isOutput=False)
    wk = nc.declare_dram_parameter("wk", [DIM, SD], BF16, isOutput=False)
    wr = nc.declare_dram_parameter("wr", [DIM, SD], BF16, isOutput=False)
    wv = nc.declare_dram_parameter("wv", [DIM, RD], BF16, isOutput=False)
    wrk = nc.declare_dram_parameter("wrk", [D, D], F32, isOutput=False)
    wout = nc.declare_dram_parameter("wout", [SD, DIM], BF16, isOutput=False)
    outp = nc.declare_dram_parameter("outp", [N, DIM], F32, isOutput=True)
    io = (xT[:], wq[:], wk[:], wr[:], wv[:], wrk[:], wout[:], outp[:])
    with tile.TileContext(nc) as tc:
        with ExitStack() as ctx:
            _emit(ctx, tc, io)
    nc.compile()
    return nc


_CACHE = {}


def _get_nc():
    if "nc" not in _CACHE:
        _CACHE["nc"] = build_nc()
    return _CACHE["nc"]


def make_in_maps(x, Wsq, Wsk, Wrv, Wrq, Wrk, Wout):
    x = np.asarray(x, np.float32)
    bf = ml_dtypes.bfloat16
    in_maps = []
    for c in range(NCORES):
        b = c // 4
        s0 = 2 * (c % 4)
        sl = slice(s0 * D, (s0 + 2) * D)
        in_maps.append({
            "xT": np.ascontiguousarray(x[b].T).astype(bf),
            "wq": np.ascontiguousarray(np.asarray(Wsq, np.float32)[:, sl]).astype(bf),
            "wk": np.ascontiguousarray(np.asarray(Wsk, np.float32)[:, sl]).astype(bf),
            "wr": np.ascontiguousarray(np.asarray(Wrq, np.float32)[:, sl]).astype(bf),
            "wv": np.ascontiguousarray(np.asarray(Wrv, np.float32)).astype(bf),
            "wrk": np.ascontiguousarray(np.asarray(Wrk, np.float32)),
            "wout": np.ascontiguousarray(np.asarray(Wout, np.float32)[sl, :]).astype(bf),
        })
    return in_maps


def combine(results):
    out = np.zeros((B, N, DIM), np.float32)
    for c in range(NCORES):
        out[c // 4] += np.asarray(results[c]["outp"], np.float32)
    return out


def kernel(x, Wsq, Wsk, Wrv, Wrq, Wrk, Wout):
    nc = _get_nc()
    in_maps = make_in_maps(x, Wsq, Wsk, Wrv, Wrq, Wrk, Wout)
    res = run_bass_kernel_spmd(nc, in_maps, list(range(NCORES))).results
    return combine(res)


def _install_ntff_shim():
    """Provide antenv.axon_hooks in images that lack it, driving NTFF
    profiling via ctypes into the injected libaxon_pjrt.so."""
    import types
    import ctypes
    import contextlib

    try:
        from antenv.axon_hooks import get_axon_ntff_profile_hook  # noqa
        return
    except ImportError:
        pass
    so_path = "/opt/axon/libaxon_pjrt.so"
    lib = ctypes.CDLL(so_path)
    if not hasattr(lib, "axon_start_nrt_profile"):
        return
    lib.axon_start_nrt_profile.argtypes = [
        ctypes.POINTER(ctypes.c_int64), ctypes.c_size_t]
    lib.axon_start_nrt_profile.restype = ctypes.c_int64
    lib.axon_stop_nrt_profile.argtypes = [ctypes.c_char_p]
    lib.axon_stop_nrt_profile.restype = ctypes.c_int64

    @contextlib.contextmanager
    def _hook(output_dir, device_ids):
        import jax
        jax.devices()
        if device_ids:
            ids = (ctypes.c_int64 * len(device_ids))(*device_ids)
            rc = lib.axon_start_nrt_profile(ids, len(device_ids))
        else:
            rc = lib.axon_start_nrt_profile(None, 0)
        if rc != 0:
            raise RuntimeError(f"axon_start_nrt_profile rc={rc}")
        try:
            yield
        finally:
            n = lib.axon_stop_nrt_profile(str(output_dir).encode())
            print(f"profile: {n} file(s) written to {output_dir}")

    import antenv
    mod = types.ModuleType("antenv.axon_hooks")
    mod.get_axon_ntff_profile_hook = lambda: _hook
    mod.set_axon_ntff_profile_hook = lambda h: None
    sys.modules["antenv.axon_hooks"] = mod
    antenv.axon_hooks = mod


def run_traced(x, Wsq, Wsk, Wrv, Wrq, Wrk, Wout, **kw):
    _install_ntff_shim()
    nc = _get_nc()
    in_maps = make_in_maps(x, Wsq, Wsk, Wrv, Wrq, Wrk, Wout)
    br = run_bass_kernel_spmd(nc, in_maps, list(range(NCORES)), trace=True, **kw)
    return combine(br.results), br



# revision 2
# speedup vs baseline: 1.5632x; 1.5632x over previous
"""Compositional attention Trainium2 Bass kernel (V3: PE-dense restructure).

Sharding: 8 cores = 2 batches x 4 search-pairs.
Core c handles batch b=c//4 and searches (2*(c%4), 2*(c%4)+1); each core
produces a partial output for its 128 columns of the S*D=512 concat dim
(host sums 4 partials per batch).

V3 notes (over V2):
  - Score matmuls for the two searches are emitted adjacent with distinct
    PE row-groups ((0,0)/(64,0)) so both K=64 matmuls run concurrently.
  - x is DMA'd in 8 per-k chunks so projections start after the first
    chunk instead of stalling ~12us on the monolithic load.
  - vnat (v in [keys, r*d] layout) comes from DMA-xbar transposes instead
    of PE transposes.
  - Epilogue: d = r0-r1 / dW / dprod are computed per query block during
    attention; dW uses bf16 single-pass matmuls on diagonal PE tiles
    ((0,0) and (64,64) run concurrently); r-softmax blend uses
    comp = inv*r1 + (sig*inv)*(r0-r1) so only two [64,N] broadcasts
    (bf16, via DRAM) are needed per search.
  - All [64/128, N] elementwise work is bf16 (2x DVE mode); output is
    written bf16 and upcast on the host.
"""

import sys

for _p in ("/opt/trn_rl_repo",):
    if _p not in sys.path:
        sys.path.insert(0, _p)

from contextlib import ExitStack

import ml_dtypes
import numpy as np

import concourse.bass as bass
import concourse.tile as tile
from concourse import bacc
from concourse import mybir
from concourse.bass import ts
from concourse.bass_utils import run_bass_kernel_spmd

B, N, DIM, S, R, D = 2, 2048, 1024, 8, 2, 64
NCORES = 8
SPC = 2          # searches per core
SD = SPC * D     # 128 (per-core slice of S*D)
RD = R * D       # 128
P = 128
IBL = 512        # query block
NIB = N // IBL   # 4
KC = DIM // P    # 8
NJT = N // P     # 16 key tiles
F32 = mybir.dt.float32
BF16 = mybir.dt.bfloat16
SCALE = float(D) ** -0.5
AF = mybir.ActivationFunctionType
ALU = mybir.AluOpType


def _emit(ctx: ExitStack, tc: tile.TileContext, io):
    nc = tc.nc
    xT, wq, wk, wr, wv, wrk, wout, outp = io

    singles = ctx.enter_context(tc.tile_pool(name="singles", bufs=1))
    ones_b = singles.tile([P, 1], BF16)
    nc.vector.memset(ones_b, 1.0)

    wq_sb = singles.tile([P, KC, SD], BF16)
    wk_sb = singles.tile([P, KC, SD], BF16)
    wr_sb = singles.tile([P, KC, SD], BF16)
    wv_sb = singles.tile([P, KC, RD], BF16)
    for dst, src in ((wq_sb, wq), (wk_sb, wk), (wr_sb, wr), (wv_sb, wv)):
        nc.sync.dma_start(out=dst, in_=src.rearrange("(kc p) m -> p kc m", p=P))
    wrk2 = singles.tile([P, D], BF16)   # Wrk twice: rows 0:64 and 64:128
    nc.sync.dma_start(out=wrk2[0:64, :], in_=wrk)
    nc.sync.dma_start(out=wrk2[64:128, :], in_=wrk)
    wout_sb = singles.tile([P, DIM], BF16)
    nc.sync.dma_start(out=wout_sb, in_=wout)

    acts = ctx.enter_context(tc.tile_pool(name="acts", bufs=1))
    qT = acts.tile([P, N], BF16)
    kT = acts.tile([P, N], BF16)
    rqT = acts.tile([P, N], BF16)
    vnat = acts.tile([P, NJT, RD], BF16)   # [key-part, key-tile, r*d]
    ret0 = acts.tile([P, N], BF16)         # search0 retrievedT (unnormalized)
    ret1 = acts.tile([P, N], BF16)         # search1
    rsh = acts.tile([P, N], BF16)          # [0:64]=s0 r1 shifted dn, [64:128]=s1 r0 up
    dT = acts.tile([P, N], BF16)           # r0-r1 per search (s0 rows 0:64)
    dprod = acts.tile([P, N], BF16)        # rq * (d @ Wrk)
    red0 = acts.tile([P, N], BF16)         # per-key-part exp sums
    red1 = acts.tile([P, N], BF16)
    bc0 = acts.tile([P, N], BF16)          # broadcast sig*inv
    bc1 = acts.tile([P, N], BF16)          # broadcast inv
    comp = acts.tile([P, N], BF16)
    rets = (ret0, ret1)
    reds = (red0, red1)

    # ---------------- projections ----------------
    with tc.tile_pool(name="xpool", bufs=1) as xpool, \
         tc.tile_pool(name="pja", bufs=1, space="PSUM") as pja, \
         tc.tile_pool(name="pjb", bufs=2, space="PSUM") as pjb:
        xs = xpool.tile([P, KC, N], BF16)
        for k in range(KC):
            nc.sync.dma_start(out=xs[:, k, :],
                              in_=xT.rearrange("(kc p) n -> p kc n", p=P)[:, k, :])
        vtmp = xpool.tile([P, N], BF16)
        # pass A: kT + vT (8 banks), k-ordered so MM k waits only chunk k
        kps = [pja.tile([P, IBL], F32, tag="pk", name=f"pk{ib}")
               for ib in range(NIB)]
        vps = [pja.tile([P, IBL], F32, tag="pv", name=f"pv{ib}")
               for ib in range(NIB)]
        for k in range(KC):
            for ib in range(NIB):
                nc.tensor.matmul(kps[ib], lhsT=wk_sb[:, k, :],
                                 rhs=xs[:, k, ts(ib, IBL)],
                                 start=(k == 0), stop=(k == KC - 1))
                nc.tensor.matmul(vps[ib], lhsT=wv_sb[:, k, :],
                                 rhs=xs[:, k, ts(ib, IBL)],
                                 start=(k == 0), stop=(k == KC - 1))
        for ib in range(NIB):
            nc.vector.tensor_copy(out=kT[:, ts(ib, IBL)], in_=kps[ib])
            nc.scalar.copy(out=vtmp[:, ts(ib, IBL)], in_=vps[ib])
            for h in range(IBL // P):
                jt = ib * (IBL // P) + h
                nc.sync.dma_start_transpose(vnat[:, jt, :], vtmp[:, ts(jt, P)])
        # pass B: qT + rqT, ib-ordered so attention ib0 can start early
        for ib in range(NIB):
            qp = pjb.tile([P, IBL], F32, tag="pq", name="pq")
            rp = pjb.tile([P, IBL], F32, tag="pr", name="pr")
            for k in range(KC):
                nc.tensor.matmul(qp, lhsT=wq_sb[:, k, :],
                                 rhs=xs[:, k, ts(ib, IBL)],
                                 start=(k == 0), stop=(k == KC - 1))
                nc.tensor.matmul(rp, lhsT=wr_sb[:, k, :],
                                 rhs=xs[:, k, ts(ib, IBL)],
                                 start=(k == 0), stop=(k == KC - 1))
            nc.vector.tensor_copy(out=qT[:, ts(ib, IBL)], in_=qp)
            nc.scalar.copy(out=rqT[:, ts(ib, IBL)], in_=rp)

    # DRAM bounce buffers for per-query scalars
    dramp = ctx.enter_context(tc.tile_pool(name="dramp", bufs=1, space="DRAM"))
    sums_dr = [dramp.tile([N], F32, tag=f"sums{si}", name=f"sums{si}")
               for si in range(SPC)]
    diff_dr = [dramp.tile([N], F32, tag=f"diff{si}", name=f"diff{si}")
               for si in range(SPC)]
    a0_dr = [dramp.tile([N], BF16, tag=f"a0{si}", name=f"a0d{si}")
             for si in range(SPC)]
    a1_dr = [dramp.tile([N], BF16, tag=f"a1{si}", name=f"a1d{si}")
             for si in range(SPC)]

    # ---------------- attention ----------------
    with tc.tile_pool(name="expp", bufs=2) as expp, \
         tc.tile_pool(name="trp1", bufs=2) as trp1, \
         tc.tile_pool(name="trp2", bufs=2) as trp2, \
         tc.tile_pool(name="trp3", bufs=2) as trp3, \
         tc.tile_pool(name="scp", bufs=2, space="PSUM") as scp, \
         tc.tile_pool(name="retp", bufs=1, space="PSUM") as retp, \
         tc.tile_pool(name="dwp", bufs=2, space="PSUM") as dwp:
        for ib in range(NIB):
            ets = [expp.tile([P, NJT, IBL], BF16, tag="exp", name=f"exp{si}")
                   for si in range(SPC)]
            rt = [retp.tile([P, IBL], F32, tag=f"rt{si}", name=f"rt{si}")
                  for si in range(SPC)]
            for jg in range(NJT // 2):
                sps = [scp.tile([P, 2, IBL], F32, tag="sc", name=f"sc{si}")
                       for si in range(SPC)]
                # 4 score MMs adjacent, alternating row groups (0,0)/(64,0)
                # so the two K=64 matmuls run concurrently on the PE
                for h in range(2):
                    jt = 2 * jg + h
                    for si in range(SPC):
                        lo = 64 * si
                        nc.tensor.matmul(
                            sps[si][:, h, :],
                            lhsT=kT[lo:lo + 64, ts(jt, P)],
                            rhs=qT[lo:lo + 64, ts(ib, IBL)],
                            start=True, stop=True,
                        )
                for si in range(SPC):
                    nc.scalar.activation(
                        out=ets[si][:, ts(jg, 2), :], in_=sps[si],
                        func=AF.Exp, scale=SCALE,
                    )
                for h in range(2):
                    jt = 2 * jg + h
                    for si in range(SPC):
                        nc.tensor.matmul(
                            rt[si], lhsT=vnat[:, jt, :], rhs=ets[si][:, jt, :],
                            start=(jt == 0), stop=(jt == NJT - 1),
                            skip_group_check=True,
                        )
            for si in range(SPC):
                nc.vector.tensor_copy(out=rets[si][:, ts(ib, IBL)], in_=rt[si])
            # partition shifts for the r1/r0 halves (so d/dprod stay
            # lane-aligned per search): s0 r1 -> rows 0:64, s1 r0 -> rows 64:128
            nc.gpsimd.dma_start(out=rsh[0:64, ts(ib, IBL)],
                                in_=ret0[64:128, ts(ib, IBL)])
            nc.gpsimd.dma_start(out=rsh[64:128, ts(ib, IBL)],
                                in_=ret1[0:64, ts(ib, IBL)])
            # d = r0 - r1 per search (s1's is negated: r0(shifted) - r1)
            nc.vector.tensor_tensor(dT[0:64, ts(ib, IBL)],
                                    ret0[0:64, ts(ib, IBL)],
                                    rsh[0:64, ts(ib, IBL)], ALU.subtract)
            nc.vector.tensor_tensor(dT[64:128, ts(ib, IBL)],
                                    rsh[64:128, ts(ib, IBL)],
                                    ret1[64:128, ts(ib, IBL)], ALU.subtract)
            # dW = Wrk^T @ d on diagonal PE tiles (0,0) and (64,64)
            dwps = dwp.tile([P, IBL], F32, tag="dw", name="dw")
            nc.tensor.matmul(dwps[0:64, :], lhsT=wrk2[0:64, :],
                             rhs=dT[0:64, ts(ib, IBL)], start=True, stop=True)
            nc.tensor.matmul(dwps[64:128, :], lhsT=wrk2[64:128, :],
                             rhs=dT[64:128, ts(ib, IBL)], start=True, stop=True)
            nc.vector.tensor_tensor(dprod[:, ts(ib, IBL)],
                                    rqT[:, ts(ib, IBL)], dwps, ALU.mult)
            # denominator tree: lvl1+2 DVE, lvl3+4 GpSimd
            for si in range(SPC):
                g1 = trp1.tile([P, NJT // 2, IBL], BF16, tag="g1",
                               name=f"g1_{si}")
                nc.vector.tensor_tensor(g1, ets[si][:, 0:8, :],
                                        ets[si][:, 8:16, :], ALU.add)
                g2 = trp2.tile([P, NJT // 4, IBL], BF16, tag="g2",
                               name=f"g2_{si}")
                nc.vector.tensor_tensor(g2, g1[:, 0:4, :], g1[:, 4:8, :],
                                        ALU.add)
                g3 = trp3.tile([P, NJT // 8, IBL], BF16, tag="g3",
                               name=f"g3_{si}")
                nc.gpsimd.tensor_tensor(g3, g2[:, 0:2, :], g2[:, 2:4, :],
                                        ALU.add)
                nc.gpsimd.tensor_tensor(reds[si][:, ts(ib, IBL)],
                                        g3[:, 0, :], g3[:, 1, :], ALU.add)

    # ---------------- epilogue ----------------
    eps = ctx.enter_context(tc.tile_pool(name="eps", bufs=2, space="PSUM"))
    ops = ctx.enter_context(tc.tile_pool(name="ops", bufs=2, space="PSUM"))
    etmp = ctx.enter_context(tc.tile_pool(name="etmp", bufs=2))
    otmp = ctx.enter_context(tc.tile_pool(name="otmp", bufs=2))
    for si in range(SPC):
        lo = 64 * si
        # per-query rows: sums (denominator) and diff (r-score gap)
        for ib in range(NIB):
            rows = eps.tile([1, 2, IBL], F32, tag="rows", name="rows")
            nc.tensor.matmul(rows[0:1, 0, :], lhsT=ones_b,
                             rhs=reds[si][:, ts(ib, IBL)], start=True,
                             stop=True)
            nc.tensor.matmul(rows[0:1, 1, :], lhsT=ones_b[lo:lo + 64, :],
                             rhs=dprod[lo:lo + 64, ts(ib, IBL)], start=True,
                             stop=True)
            rowsb = etmp.tile([1, 2, IBL], F32, tag="rowsb", name="rowsb")
            nc.scalar.copy(out=rowsb, in_=rows)
            nc.sync.dma_start(out=sums_dr[si][None, ts(ib, IBL)],
                              in_=rowsb[0:1, 0, :])
            nc.sync.dma_start(out=diff_dr[si][None, ts(ib, IBL)],
                              in_=rowsb[0:1, 1, :])
        # per-query scalar math in [128, 16] layout
        s128 = etmp.tile([P, N // P], F32, tag="s128")
        d128 = etmp.tile([P, N // P], F32, tag="d128")
        nc.gpsimd.dma_start(out=s128,
                            in_=sums_dr[si].rearrange("(p f) -> p f", p=P))
        nc.gpsimd.dma_start(out=d128,
                            in_=diff_dr[si].rearrange("(p f) -> p f", p=P))
        inv = etmp.tile([P, N // P], F32, tag="inv")
        nc.vector.reciprocal(inv, s128)
        t16 = etmp.tile([P, N // P], F32, tag="t16")
        nc.vector.tensor_tensor(t16, d128, inv, ALU.mult)
        ra0 = etmp.tile([P, N // P], F32, tag="ra0")
        nc.scalar.activation(out=ra0, in_=t16, func=AF.Sigmoid, scale=SCALE)
        a0b = etmp.tile([P, N // P], BF16, tag="a0b")
        nc.vector.tensor_tensor(a0b, ra0, inv, ALU.mult)
        a1b = etmp.tile([P, N // P], BF16, tag="a1b")
        nc.vector.tensor_copy(out=a1b, in_=inv)
        nc.gpsimd.dma_start(out=a0_dr[si].rearrange("(p f) -> p f", p=P),
                            in_=a0b)
        nc.gpsimd.dma_start(out=a1_dr[si].rearrange("(p f) -> p f", p=P),
                            in_=a1b)
        nc.sync.dma_start(out=bc0[lo:lo + 64, :],
                          in_=a0_dr[si][None, :].to_broadcast([64, N]))
        nc.sync.dma_start(out=bc1[lo:lo + 64, :],
                          in_=a1_dr[si][None, :].to_broadcast([64, N]))
        # comp = inv*r1 + (sig*inv)*(r0-r1); r1 is rsh rows for s0, in-place for s1
        r1ap = rsh[0:64, :] if si == 0 else ret1[64:128, :]
        t1 = etmp.tile([P, N], BF16, tag="t1", name=f"t1_{si}")
        nc.vector.tensor_tensor(t1[lo:lo + 64, :], bc0[lo:lo + 64, :],
                                dT[lo:lo + 64, :], ALU.mult)
        t2 = etmp.tile([P, N], BF16, tag="t2", name=f"t2_{si}")
        nc.gpsimd.tensor_tensor(t2[lo:lo + 64, :], bc1[lo:lo + 64, :],
                                r1ap, ALU.mult)
        nc.vector.tensor_tensor(comp[lo:lo + 64, :], t1[lo:lo + 64, :],
                                t2[lo:lo + 64, :], ALU.add)

    # ---------------- output projection ----------------
    for nch in range(N // P):
        op = ops.tile([P, 2, IBL], F32, tag="op", name="op")
        for h in range(2):
            nc.tensor.matmul(op[:, h, :], lhsT=comp[:, ts(nch, P)],
                             rhs=wout_sb[:, ts(h, IBL)], start=True, stop=True)
        osb = otmp.tile([P, DIM], BF16, tag="osb", name="osb")
        if nch % 2 == 0:
            nc.scalar.copy(out=osb, in_=op)
        else:
            nc.vector.tensor_copy(out=osb, in_=op)
        nc.sync.dma_start(out=outp[ts(nch, P), :], in_=osb)


def build_nc():
    nc = bacc.Bacc()
    xT = nc.declare_dram_parameter("xT", [DIM, N], BF16, isOutput=False)
    wq = nc.declare_dram_parameter("wq", [DIM, SD], BF16, isOutput=False)
    wk = nc.declare_dram_parameter("wk", [DIM, SD], BF16, isOutput=False)
    wr = nc.declare_dram_parameter("wr", [DIM, SD], BF16, isOutput=False)
    wv = nc.declare_dram_parameter("wv", [DIM, RD], BF16, isOutput=False)
    wrk = nc.declare_dram_parameter("wrk", [D, D], BF16, isOutput=False)
    wout = nc.declare_dram_parameter("wout", [SD, DIM], BF16, isOutput=False)
    outp = nc.declare_dram_parameter("outp", [N, DIM], BF16, isOutput=True)
    io = (xT[:], wq[:], wk[:], wr[:], wv[:], wrk[:], wout[:], outp[:])
    with tile.TileContext(nc) as tc:
        with ExitStack() as ctx:
            _emit(ctx, tc, io)
    nc.compile()
    return nc


_CACHE = {}


def _get_nc():
    if "nc" not in _CACHE:
        _CACHE["nc"] = build_nc()
    return _CACHE["nc"]


def make_in_maps(x, Wsq, Wsk, Wrv, Wrq, Wrk, Wout):
    x = np.asarray(x, np.float32)
    bf = ml_dtypes.bfloat16
    in_maps = []
    for c in range(NCORES):
        b = c // 4
        s0 = 2 * (c % 4)
        sl = slice(s0 * D, (s0 + 2) * D)
        in_maps.append({
            "xT": np.ascontiguousarray(x[b].T).astype(bf),
            "wq": np.ascontiguousarray(np.asarray(Wsq, np.float32)[:, sl]).astype(bf),
            "wk": np.ascontiguousarray(np.asarray(Wsk, np.float32)[:, sl]).astype(bf),
            "wr": np.ascontiguousarray(np.asarray(Wrq, np.float32)[:, sl]).astype(bf),
            "wv": np.ascontiguousarray(np.asarray(Wrv, np.float32)).astype(bf),
            "wrk": np.ascontiguousarray(np.asarray(Wrk, np.float32)).astype(bf),
            "wout": np.ascontiguousarray(np.asarray(Wout, np.float32)[sl, :]).astype(bf),
        })
    return in_maps


def combine(results):
    out = np.zeros((B, N, DIM), np.float32)
    for c in range(NCORES):
        out[c // 4] += np.asarray(results[c]["outp"], np.float32)
    return out


def kernel(x, Wsq, Wsk, Wrv, Wrq, Wrk, Wout):
    nc = _get_nc()
    in_maps = make_in_maps(x, Wsq, Wsk, Wrv, Wrq, Wrk, Wout)
    res = run_bass_kernel_spmd(nc, in_maps, list(range(NCORES))).results
    return combine(res)


def _install_ntff_shim():
    """Provide antenv.axon_hooks in images that lack it, driving NTFF
    profiling via ctypes into the injected libaxon_pjrt.so."""
    import types
    import ctypes
    import contextlib

    try:
        from antenv.axon_hooks import get_axon_ntff_profile_hook  # noqa
        return
    except ImportError:
        pass
    so_path = "/opt/axon/libaxon_pjrt.so"
    lib = ctypes.CDLL(so_path)
    if not hasattr(lib, "axon_start_nrt_profile"):
        return
    lib.axon_start_nrt_profile.argtypes = [
        ctypes.POINTER(ctypes.c_int64), ctypes.c_size_t]
    lib.axon_start_nrt_profile.restype = ctypes.c_int64
    lib.axon_stop_nrt_profile.argtypes = [ctypes.c_char_p]
    lib.axon_stop_nrt_profile.restype = ctypes.c_int64

    @contextlib.contextmanager
    def _hook(output_dir, device_ids):
        import jax
        jax.devices()
        if device_ids:
            ids = (ctypes.c_int64 * len(device_ids))(*device_ids)
            rc = lib.axon_start_nrt_profile(ids, len(device_ids))
        else:
            rc = lib.axon_start_nrt_profile(None, 0)
        if rc != 0:
            raise RuntimeError(f"axon_start_nrt_profile rc={rc}")
        try:
            yield
        finally:
            n = lib.axon_stop_nrt_profile(str(output_dir).encode())
            print(f"profile: {n} file(s) written to {output_dir}")

    import antenv
    mod = types.ModuleType("antenv.axon_hooks")
    mod.get_axon_ntff_profile_hook = lambda: _hook
    mod.set_axon_ntff_profile_hook = lambda h: None
    sys.modules["antenv.axon_hooks"] = mod
    antenv.axon_hooks = mod


def run_traced(x, Wsq, Wsk, Wrv, Wrq, Wrk, Wout, **kw):
    _install_ntff_shim()
    nc = _get_nc()
    in_maps = make_in_maps(x, Wsq, Wsk, Wrv, Wrq, Wrk, Wout)
    br = run_bass_kernel_spmd(nc, in_maps, list(range(NCORES)), trace=True, **kw)
    return combine(br.results), br


# revision 4
# speedup vs baseline: 1.8400x; 1.1771x over previous
"""Compositional attention Trainium2 Bass kernel (V4).

Sharding: 8 cores = 2 batches x 4 search-pairs.
Core c handles batch b=c//4 and searches (2*(c%4), 2*(c%4)+1); each core
produces a partial output for its 128 columns of the S*D=512 concat dim
(host sums 4 partials per batch).

V4 notes (over V3):
  - Per key tile, both searches' score matmuls write one [P, 2, IBL]
    PSUM tile and one exp covers both; the two K=64 scores become ready
    together and stay adjacent on the PE queue, so they execute
    concurrently in disjoint row groups ((0,0) and (64,0)).
  - x chunk DMAs go out first on the sync queue; weights ride the
    scalar queue, so the first projection matmul starts ~9us earlier
    and the PE warms up sooner.
  - Epilogue per-search chains are interleaved step-by-step across
    engines/queues instead of running serially.
"""

import sys

for _p in ("/opt/trn_rl_repo",):
    if _p not in sys.path:
        sys.path.insert(0, _p)

from contextlib import ExitStack

import ml_dtypes
import numpy as np

import concourse.bass as bass
import concourse.tile as tile
from concourse import bacc
from concourse import mybir
from concourse.bass import ts
from concourse.bass_utils import run_bass_kernel_spmd

B, N, DIM, S, R, D = 2, 2048, 1024, 8, 2, 64
NCORES = 8
SPC = 2          # searches per core
SD = SPC * D     # 128 (per-core slice of S*D)
RD = R * D       # 128
P = 128
IBL = 512        # query block
NIB = N // IBL   # 4
KC = DIM // P    # 8
NJT = N // P     # 16 key tiles
F32 = mybir.dt.float32
BF16 = mybir.dt.bfloat16
SCALE = float(D) ** -0.5
AF = mybir.ActivationFunctionType
ALU = mybir.AluOpType


def _emit(ctx: ExitStack, tc: tile.TileContext, io):
    nc = tc.nc
    xT, wq, wk, wr, wv, wrk, wout, outp = io

    singles = ctx.enter_context(tc.tile_pool(name="singles", bufs=1))
    ones_b = singles.tile([P, 1], BF16)
    nc.vector.memset(ones_b, 1.0)

    wq_sb = singles.tile([P, KC, SD], BF16)
    wk_sb = singles.tile([P, KC, SD], BF16)
    wr_sb = singles.tile([P, KC, SD], BF16)
    wv_sb = singles.tile([P, KC, RD], BF16)
    wrk2 = singles.tile([P, D], BF16)   # Wrk twice: rows 0:64 and 64:128
    wout_sb = singles.tile([P, DIM], BF16)

    acts = ctx.enter_context(tc.tile_pool(name="acts", bufs=1))
    qT = acts.tile([P, N], BF16)
    kT = acts.tile([P, N], BF16)
    rqT = acts.tile([P, N], BF16)
    vnat = acts.tile([P, NJT, RD], BF16)   # [key-part, key-tile, r*d]
    ret0 = acts.tile([P, N], BF16)         # search0 retrievedT (unnormalized)
    ret1 = acts.tile([P, N], BF16)         # search1
    rsh = acts.tile([P, N], BF16)          # [0:64]=s0 r1 shifted dn, [64:128]=s1 r0 up
    dT = acts.tile([P, N], BF16)           # r0-r1 per search (s0 rows 0:64)
    dprod = acts.tile([P, N], BF16)        # rq * (d @ Wrk)
    red0 = acts.tile([P, N], BF16)         # per-key-part exp sums
    red1 = acts.tile([P, N], BF16)
    bc0 = acts.tile([P, N], BF16)          # broadcast sig*inv
    bc1 = acts.tile([P, N], BF16)          # broadcast inv
    comp = acts.tile([P, N], BF16)
    rets = (ret0, ret1)
    reds = (red0, red1)

    # ---------------- projections ----------------
    with tc.tile_pool(name="xpool", bufs=1) as xpool, \
         tc.tile_pool(name="pja", bufs=1, space="PSUM") as pja, \
         tc.tile_pool(name="pjb", bufs=2, space="PSUM") as pjb:
        xs = xpool.tile([P, KC, N], BF16)
        xr = xT.rearrange("(kc p) n -> p kc n", p=P)
        # x chunk 0 first (unblocks the first matmuls), weights on the
        # scalar DMA queue so they don't delay the x stream
        nc.sync.dma_start(out=xs[:, 0, :], in_=xr[:, 0, :])
        nc.scalar.dma_start(out=wk_sb,
                            in_=wk.rearrange("(kc p) m -> p kc m", p=P))
        nc.scalar.dma_start(out=wv_sb,
                            in_=wv.rearrange("(kc p) m -> p kc m", p=P))
        for k in range(1, KC):
            nc.sync.dma_start(out=xs[:, k, :], in_=xr[:, k, :])
        nc.scalar.dma_start(out=wq_sb,
                            in_=wq.rearrange("(kc p) m -> p kc m", p=P))
        nc.scalar.dma_start(out=wr_sb,
                            in_=wr.rearrange("(kc p) m -> p kc m", p=P))
        nc.scalar.dma_start(out=wrk2[0:64, :], in_=wrk)
        nc.scalar.dma_start(out=wrk2[64:128, :], in_=wrk)
        nc.scalar.dma_start(out=wout_sb, in_=wout)

        vtmp = xpool.tile([P, N], BF16)
        # pass A: kT + vT (8 banks), k-ordered so MM k waits only chunk k
        kps = [pja.tile([P, IBL], F32, tag="pk", name=f"pk{ib}")
               for ib in range(NIB)]
        vps = [pja.tile([P, IBL], F32, tag="pv", name=f"pv{ib}")
               for ib in range(NIB)]
        for k in range(KC):
            for ib in range(NIB):
                nc.tensor.matmul(kps[ib], lhsT=wk_sb[:, k, :],
                                 rhs=xs[:, k, ts(ib, IBL)],
                                 start=(k == 0), stop=(k == KC - 1))
                nc.tensor.matmul(vps[ib], lhsT=wv_sb[:, k, :],
                                 rhs=xs[:, k, ts(ib, IBL)],
                                 start=(k == 0), stop=(k == KC - 1))
        for ib in range(NIB):
            nc.vector.tensor_copy(out=kT[:, ts(ib, IBL)], in_=kps[ib])
            nc.scalar.copy(out=vtmp[:, ts(ib, IBL)], in_=vps[ib])
            for h in range(IBL // P):
                jt = ib * (IBL // P) + h
                nc.scalar.dma_start_transpose(vnat[:, jt, :], vtmp[:, ts(jt, P)])
        # pass B: qT + rqT, ib-ordered so attention ib0 can start early
        for ib in range(NIB):
            qp = pjb.tile([P, IBL], F32, tag="pq", name="pq")
            rp = pjb.tile([P, IBL], F32, tag="pr", name="pr")
            for k in range(KC):
                nc.tensor.matmul(qp, lhsT=wq_sb[:, k, :],
                                 rhs=xs[:, k, ts(ib, IBL)],
                                 start=(k == 0), stop=(k == KC - 1))
                nc.tensor.matmul(rp, lhsT=wr_sb[:, k, :],
                                 rhs=xs[:, k, ts(ib, IBL)],
                                 start=(k == 0), stop=(k == KC - 1))
            nc.vector.tensor_copy(out=qT[:, ts(ib, IBL)], in_=qp)
            nc.scalar.copy(out=rqT[:, ts(ib, IBL)], in_=rp)

    # DRAM bounce buffers for per-query scalars
    dramp = ctx.enter_context(tc.tile_pool(name="dramp", bufs=1, space="DRAM"))
    sums_dr = [dramp.tile([N], F32, tag=f"sums{si}", name=f"sums{si}")
               for si in range(SPC)]
    diff_dr = [dramp.tile([N], F32, tag=f"diff{si}", name=f"diff{si}")
               for si in range(SPC)]
    a0_dr = [dramp.tile([N], BF16, tag=f"a0{si}", name=f"a0d{si}")
             for si in range(SPC)]
    a1_dr = [dramp.tile([N], BF16, tag=f"a1{si}", name=f"a1d{si}")
             for si in range(SPC)]

    # ---------------- attention ----------------
    with tc.tile_pool(name="expp", bufs=2) as expp, \
         tc.tile_pool(name="trp1", bufs=2) as trp1, \
         tc.tile_pool(name="trp2", bufs=2) as trp2, \
         tc.tile_pool(name="trp3", bufs=2) as trp3, \
         tc.tile_pool(name="scp", bufs=2, space="PSUM") as scp, \
         tc.tile_pool(name="retp", bufs=1, space="PSUM") as retp, \
         tc.tile_pool(name="dwp", bufs=2, space="PSUM") as dwp:
        for ib in range(NIB):
            # exp tiles: [part, key-tile, search, queries]
            ets = expp.tile([P, NJT, SPC, IBL], BF16, tag="exp", name="exp")
            rt = [retp.tile([P, IBL], F32, tag=f"rt{si}", name=f"rt{si}")
                  for si in range(SPC)]
            for jt in range(NJT):
                sp = scp.tile([P, SPC, IBL], F32, tag="sc", name="sc")
                # both searches' scores for this key tile: adjacent on the
                # PE queue, disjoint row groups -> run concurrently; one
                # exp frees both banks together so the pairing persists
                for si in range(SPC):
                    lo = 64 * si
                    nc.tensor.matmul(
                        sp[:, si, :],
                        lhsT=kT[lo:lo + 64, ts(jt, P)],
                        rhs=qT[lo:lo + 64, ts(ib, IBL)],
                        start=True, stop=True,
                    )
                nc.scalar.activation(out=ets[:, jt, :, :], in_=sp,
                                     func=AF.Exp, scale=SCALE)
                for si in range(SPC):
                    nc.tensor.matmul(
                        rt[si], lhsT=vnat[:, jt, :], rhs=ets[:, jt, si, :],
                        start=(jt == 0), stop=(jt == NJT - 1),
                        skip_group_check=True,
                    )
            for si in range(SPC):
                nc.vector.tensor_copy(out=rets[si][:, ts(ib, IBL)], in_=rt[si])
            # partition shifts for the r1/r0 halves (so d/dprod stay
            # lane-aligned per search): s0 r1 -> rows 0:64, s1 r0 -> rows 64:128
            nc.gpsimd.dma_start(out=rsh[0:64, ts(ib, IBL)],
                                in_=ret0[64:128, ts(ib, IBL)])
            nc.gpsimd.dma_start(out=rsh[64:128, ts(ib, IBL)],
                                in_=ret1[0:64, ts(ib, IBL)])
            # d = r0 - r1 per search (s1's r0 is the shifted copy)
            nc.vector.tensor_tensor(dT[0:64, ts(ib, IBL)],
                                    ret0[0:64, ts(ib, IBL)],
                                    rsh[0:64, ts(ib, IBL)], ALU.subtract)
            nc.vector.tensor_tensor(dT[64:128, ts(ib, IBL)],
                                    rsh[64:128, ts(ib, IBL)],
                                    ret1[64:128, ts(ib, IBL)], ALU.subtract)
            # dW = Wrk^T @ d on diagonal PE tiles (0,0) and (64,64)
            dwps = dwp.tile([P, IBL], F32, tag="dw", name="dw")
            nc.tensor.matmul(dwps[0:64, :], lhsT=wrk2[0:64, :],
                             rhs=dT[0:64, ts(ib, IBL)], start=True, stop=True)
            nc.tensor.matmul(dwps[64:128, :], lhsT=wrk2[64:128, :],
                             rhs=dT[64:128, ts(ib, IBL)], start=True, stop=True)
            nc.vector.tensor_tensor(dprod[:, ts(ib, IBL)],
                                    rqT[:, ts(ib, IBL)], dwps, ALU.mult)
            # denominator tree: lvl1+2 DVE, lvl3+4 GpSimd
            for si in range(SPC):
                g1 = trp1.tile([P, NJT // 2, IBL], BF16, tag="g1",
                               name=f"g1_{si}")
                nc.vector.tensor_tensor(g1, ets[:, 0:8, si, :],
                                        ets[:, 8:16, si, :], ALU.add)
                g2 = trp2.tile([P, NJT // 4, IBL], BF16, tag="g2",
                               name=f"g2_{si}")
                nc.vector.tensor_tensor(g2, g1[:, 0:4, :], g1[:, 4:8, :],
                                        ALU.add)
                g3 = trp3.tile([P, NJT // 8, IBL], BF16, tag="g3",
                               name=f"g3_{si}")
                nc.gpsimd.tensor_tensor(g3, g2[:, 0:2, :], g2[:, 2:4, :],
                                        ALU.add)
                nc.gpsimd.tensor_tensor(reds[si][:, ts(ib, IBL)],
                                        g3[:, 0, :], g3[:, 1, :], ALU.add)

    # ---------------- epilogue ----------------
    eps = ctx.enter_context(tc.tile_pool(name="eps", bufs=2, space="PSUM"))
    ops = ctx.enter_context(tc.tile_pool(name="ops", bufs=2, space="PSUM"))
    etmp = ctx.enter_context(tc.tile_pool(name="etmp", bufs=2))
    otmp = ctx.enter_context(tc.tile_pool(name="otmp", bufs=2))
    # per-query rows: sums (denominator) and diff (r-score gap)
    for ib in range(NIB):
        for si in range(SPC):
            lo = 64 * si
            rows = eps.tile([1, 2, IBL], F32, tag="rows", name="rows")
            nc.tensor.matmul(rows[0:1, 0, :], lhsT=ones_b,
                             rhs=reds[si][:, ts(ib, IBL)], start=True,
                             stop=True)
            nc.tensor.matmul(rows[0:1, 1, :], lhsT=ones_b[lo:lo + 64, :],
                             rhs=dprod[lo:lo + 64, ts(ib, IBL)], start=True,
                             stop=True)
            rowsb = etmp.tile([1, 2, IBL], F32, tag="rowsb", name="rowsb")
            if si == 0:
                nc.scalar.copy(out=rowsb, in_=rows)
                nc.sync.dma_start(out=sums_dr[si][None, ts(ib, IBL)],
                                  in_=rowsb[0:1, 0, :])
                nc.sync.dma_start(out=diff_dr[si][None, ts(ib, IBL)],
                                  in_=rowsb[0:1, 1, :])
            else:
                nc.vector.tensor_copy(out=rowsb, in_=rows)
                nc.scalar.dma_start(out=sums_dr[si][None, ts(ib, IBL)],
                                    in_=rowsb[0:1, 0, :])
                nc.scalar.dma_start(out=diff_dr[si][None, ts(ib, IBL)],
                                    in_=rowsb[0:1, 1, :])
    # per-query scalar math in [128, 16] layout, both chains interleaved
    s128 = [etmp.tile([P, N // P], F32, tag=f"s128_{si}", name=f"s128_{si}")
            for si in range(SPC)]
    d128 = [etmp.tile([P, N // P], F32, tag=f"d128_{si}", name=f"d128_{si}")
            for si in range(SPC)]
    inv = [etmp.tile([P, N // P], F32, tag=f"inv{si}", name=f"inv{si}")
           for si in range(SPC)]
    t16 = [etmp.tile([P, N // P], F32, tag=f"t16_{si}", name=f"t16_{si}")
           for si in range(SPC)]
    ra0 = [etmp.tile([P, N // P], F32, tag=f"ra0_{si}", name=f"ra0_{si}")
           for si in range(SPC)]
    a0b = [etmp.tile([P, N // P], BF16, tag=f"a0b{si}", name=f"a0b{si}")
           for si in range(SPC)]
    a1b = [etmp.tile([P, N // P], BF16, tag=f"a1b{si}", name=f"a1b{si}")
           for si in range(SPC)]
    dmaq = (nc.sync, nc.scalar)
    for si in range(SPC):
        dmaq[si].dma_start(out=s128[si],
                           in_=sums_dr[si].rearrange("(p f) -> p f", p=P))
        dmaq[si].dma_start(out=d128[si],
                           in_=diff_dr[si].rearrange("(p f) -> p f", p=P))
    for si in range(SPC):
        nc.vector.reciprocal(inv[si], s128[si])
    for si in range(SPC):
        nc.vector.tensor_tensor(t16[si], d128[si], inv[si], ALU.mult)
        nc.scalar.activation(out=ra0[si], in_=t16[si], func=AF.Sigmoid,
                             scale=SCALE)
        nc.vector.tensor_tensor(a0b[si], ra0[si], inv[si], ALU.mult)
        nc.vector.tensor_copy(out=a1b[si], in_=inv[si])
        dmaq[si].dma_start(out=a0_dr[si].rearrange("(p f) -> p f", p=P),
                           in_=a0b[si])
        dmaq[si].dma_start(out=a1_dr[si].rearrange("(p f) -> p f", p=P),
                           in_=a1b[si])
    for si in range(SPC):
        lo = 64 * si
        dmaq[si].dma_start(out=bc0[lo:lo + 64, :],
                           in_=a0_dr[si][None, :].to_broadcast([64, N]))
        nc.gpsimd.dma_start(out=bc1[lo:lo + 64, :],
                            in_=a1_dr[si][None, :].to_broadcast([64, N]))
    # comp = inv*r1 + (sig*inv)*(r0-r1); r1 is rsh rows for s0, in-place for s1
    t1 = etmp.tile([P, N], BF16, tag="t1")
    t2 = etmp.tile([P, N], BF16, tag="t2")
    for si in range(SPC):
        lo = 64 * si
        r1ap = rsh[0:64, :] if si == 0 else ret1[64:128, :]
        if si == 0:
            nc.vector.tensor_tensor(t1[lo:lo + 64, :], bc0[lo:lo + 64, :],
                                    dT[lo:lo + 64, :], ALU.mult)
            nc.gpsimd.tensor_tensor(t2[lo:lo + 64, :], bc1[lo:lo + 64, :],
                                    r1ap, ALU.mult)
        else:
            nc.gpsimd.tensor_tensor(t1[lo:lo + 64, :], bc0[lo:lo + 64, :],
                                    dT[lo:lo + 64, :], ALU.mult)
            nc.vector.tensor_tensor(t2[lo:lo + 64, :], bc1[lo:lo + 64, :],
                                    r1ap, ALU.mult)
        nc.vector.tensor_tensor(comp[lo:lo + 64, :], t1[lo:lo + 64, :],
                                t2[lo:lo + 64, :], ALU.add)

    # ---------------- output projection ----------------
    for nch in range(N // P):
        op = ops.tile([P, 2, IBL], F32, tag="op", name="op")
        for h in range(2):
            nc.tensor.matmul(op[:, h, :], lhsT=comp[:, ts(nch, P)],
                             rhs=wout_sb[:, ts(h, IBL)], start=True, stop=True)
        osb = otmp.tile([P, DIM], BF16, tag="osb", name="osb")
        if nch % 2 == 0:
            nc.scalar.copy(out=osb, in_=op)
        else:
            nc.vector.tensor_copy(out=osb, in_=op)
        nc.sync.dma_start(out=outp[ts(nch, P), :], in_=osb)


def build_nc():
    nc = bacc.Bacc()
    xT = nc.declare_dram_parameter("xT", [DIM, N], BF16, isOutput=False)
    wq = nc.declare_dram_parameter("wq", [DIM, SD], BF16, isOutput=False)
    wk = nc.declare_dram_parameter("wk", [DIM, SD], BF16, isOutput=False)
    wr = nc.declare_dram_parameter("wr", [DIM, SD], BF16, isOutput=False)
    wv = nc.declare_dram_parameter("wv", [DIM, RD], BF16, isOutput=False)
    wrk = nc.declare_dram_parameter("wrk", [D, D], BF16, isOutput=False)
    wout = nc.declare_dram_parameter("wout", [SD, DIM], BF16, isOutput=False)
    outp = nc.declare_dram_parameter("outp", [N, DIM], BF16, isOutput=True)
    io = (xT[:], wq[:], wk[:], wr[:], wv[:], wrk[:], wout[:], outp[:])
    with tile.TileContext(nc) as tc:
        with ExitStack() as ctx:
            _emit(ctx, tc, io)
    nc.compile()
    return nc


_CACHE = {}


def _get_nc():
    if "nc" not in _CACHE:
        _CACHE["nc"] = build_nc()
    return _CACHE["nc"]


def make_in_maps(x, Wsq, Wsk, Wrv, Wrq, Wrk, Wout):
    x = np.asarray(x, np.float32)
    bf = ml_dtypes.bfloat16
    in_maps = []
    for c in range(NCORES):
        b = c // 4
        s0 = 2 * (c % 4)
        sl = slice(s0 * D, (s0 + 2) * D)
        in_maps.append({
            "xT": np.ascontiguousarray(x[b].T).astype(bf),
            "wq": np.ascontiguousarray(np.asarray(Wsq, np.float32)[:, sl]).astype(bf),
            "wk": np.ascontiguousarray(np.asarray(Wsk, np.float32)[:, sl]).astype(bf),
            "wr": np.ascontiguousarray(np.asarray(Wrq, np.float32)[:, sl]).astype(bf),
            "wv": np.ascontiguousarray(np.asarray(Wrv, np.float32)).astype(bf),
            "wrk": np.ascontiguousarray(np.asarray(Wrk, np.float32)).astype(bf),
            "wout": np.ascontiguousarray(np.asarray(Wout, np.float32)[sl, :]).astype(bf),
        })
    return in_maps


def combine(results):
    out = np.zeros((B, N, DIM), np.float32)
    for c in range(NCORES):
        out[c // 4] += np.asarray(results[c]["outp"], np.float32)
    return out


def kernel(x, Wsq, Wsk, Wrv, Wrq, Wrk, Wout):
    nc = _get_nc()
    in_maps = make_in_maps(x, Wsq, Wsk, Wrv, Wrq, Wrk, Wout)
    res = run_bass_kernel_spmd(nc, in_maps, list(range(NCORES))).results
    return combine(res)


def _install_ntff_shim():
    """Provide antenv.axon_hooks in images that lack it, driving NTFF
    profiling via ctypes into the injected libaxon_pjrt.so."""
    import types
    import ctypes
    import contextlib

    try:
        from antenv.axon_hooks import get_axon_ntff_profile_hook  # noqa
        return
    except ImportError:
        pass
    so_path = "/opt/axon/libaxon_pjrt.so"
    lib = ctypes.CDLL(so_path)
    if not hasattr(lib, "axon_start_nrt_profile"):
        return
    lib.axon_start_nrt_profile.argtypes = [
        ctypes.POINTER(ctypes.c_int64), ctypes.c_size_t]
    lib.axon_start_nrt_profile.restype = ctypes.c_int64
    lib.axon_stop_nrt_profile.argtypes = [ctypes.c_char_p]
    lib.axon_stop_nrt_profile.restype = ctypes.c_int64

    @contextlib.contextmanager
    def _hook(output_dir, device_ids):
        import jax
        jax.devices()
        if device_ids:
            ids = (ctypes.c_int64 * len(device_ids))(*device_ids)
            rc = lib.axon_start_nrt_profile(ids, len(device_ids))
        else:
            rc = lib.axon_start_nrt_profile(None, 0)
        if rc != 0:
            raise RuntimeError(f"axon_start_nrt_profile rc={rc}")
        try:
            yield
        finally:
            n = lib.axon_stop_nrt_profile(str(output_dir).encode())
            print(f"profile: {n} file(s) written to {output_dir}")

    import antenv
    mod = types.ModuleType("antenv.axon_hooks")
    mod.get_axon_ntff_profile_hook = lambda: _hook
    mod.set_axon_ntff_profile_hook = lambda h: None
    sys.modules["antenv.axon_hooks"] = mod
    antenv.axon_hooks = mod


def run_traced(x, Wsq, Wsk, Wrv, Wrq, Wrk, Wout, **kw):
    _install_ntff_shim()
    nc = _get_nc()
    in_maps = make_in_maps(x, Wsq, Wsk, Wrv, Wrq, Wrk, Wout)
    br = run_bass_kernel_spmd(nc, in_maps, list(range(NCORES)), trace=True, **kw)
    return combine(br.results), br
